# revision 27
# baseline (speedup 1.0000x reference)
"""Trainium2 Bass kernel for nn_EntropyLoss (retrieval_knn).

Computes var([E(f1)-E(f0), E(f2)-E(f1)], ddof=1) where
E(f) = log(1 + sum_b sum_i r_ball[b, i]) and r_ball[b, i] is the K-th
nearest-neighbor distance (K = C//10 = 51, i.e. 52nd smallest including
the self-distance 0) among the C=512 channel vectors (dim H*W = 4096)
of sample b.

Strategy (8 NeuronCores, data-parallel over the 48 (tensor, sample)
units, 6 units per core):
  host:   pre-transpose each unit to X^T [4096, 512] in the PE-friendly
          [128, 32, 512] chunk layout, cast to fp16 (error on the Gram
          matrix ~2e-2 against a d2 spread of ~500 -- negligible), and
          precompute sq[c] = ||x_c||^2 in fp64.
  device: per 128-row block, PSUM accumulates m = G - sq_j/2 + 2048 in
          fp32 via 1 + 32 matmuls: a K=1 "bias row" (ones^T @ fp16(2048
          - sq/2)) followed by the 32 fp16 Gram k-chunks. m is a per-row
          ranking proxy for -d2/2 (d2_ij = sq_i + sq_j - 2 G_ij =
          sq_i + 4096 - 2 m_ij; sq_i is constant per row, so max_j m
          <=> min_j d2). ScalarE copies m PSUM->SBUF; VectorE extracts
          the 52nd-largest m per row with 7 rounds of max8 +
          match_replace8 (13 passes, the DVE-bound critical path:
          max8/match_replace run at 1x mode ~760 ns/pass regardless of
          dtype -- measured on HW).
  host:   d2 = sq_i + 4096 - 2 m_sel, r = sqrt(max(d2, 0)), then the
          scalar log/var tail in fp64.

Measured on HW (device-For_i loop slope, 8 cores in parallel):
~249 us steady-state per pipeline; engine rates: DVE 13x24 selection
passes ~236 us (bound), PE 792 fp16 matmuls ~198 us, DMA 25.2 MB
~72 us (all overlapped).
"""
import sys

for _p in ("/opt/trn_rl_repo", "/root/.axon_site/_ro/trn_rl_repo"):
    if _p not in sys.path:
        sys.path.insert(0, _p)

import numpy as np

from concourse import bacc, mybir
from concourse.tile import TileContext
from concourse.bass_utils import run_bass_kernel_spmd

B, C, H, W = 16, 512, 64, 64
D = H * W  # 4096
K = C // 10  # 51 -> want 52nd smallest distance per row
RANK = K + 1  # 52
N_CORES = 8
N_TENSORS = 3
UNITS = N_TENSORS * B  # 48
UPC = UNITS // N_CORES  # units per core = 6
KCHUNKS = D // 128  # 32
RBLK = C // 128  # 4 row blocks per unit
NBLK = UPC * RBLK  # 24 blocks per core
ROUNDS = RANK // 8 + (1 if RANK % 8 else 0)  # 7
SEL_COL = (RANK - 1) % 8  # 3: index of rank-52 within round 7's top-8

TRACE = False  # test.py flips this for profiling
_LAST = {}  # debug stash


DMA_SPLIT = 4  # xt DMAs per sample (lets PE start on the first chunk early)


def _build_program(repeat=1, ablate=(), loop_n=None):
    """ablate: subset of {"sel", "mm", "dma"} for timing ablations.
    loop_n: if set, wrap the whole pipeline in a hardware For_i loop of
    that many iterations (device-side repetition for timing)."""
    nc = bacc.Bacc("TRN2", target_bir_lowering=False, debug=False)

    xt_d = nc.dram_tensor(
        "xt", [UPC, 128, KCHUNKS * C], mybir.dt.float16, kind="ExternalInput"
    )
    # sqn[s, j] = fp16(2048 - sq[s, j]/2): folded into the Gram matmul as an
    # extra K=1 accumulation row, so m = G - sq_j/2 + 2048 lands in PSUM with
    # no vector-engine subtract.
    sqn_d = nc.dram_tensor("sqn", [UPC, C], mybir.dt.float16, kind="ExternalInput")
    msel_d = nc.dram_tensor(
        "msel", [128, NBLK * 8], mybir.dt.float32, kind="ExternalOutput"
    )

    kper = KCHUNKS // DMA_SPLIT  # k-chunks per DMA piece
    xt_view = xt_d.ap().rearrange(
        "s p (d k c) -> s p d k c", d=DMA_SPLIT, k=kper
    )

    with TileContext(nc) as tc:
        with (
            tc.tile_pool(name="xpool", bufs=2 * DMA_SPLIT) as xpool,
            tc.tile_pool(name="small", bufs=2) as small,
            tc.tile_pool(name="consts", bufs=1) as consts,
            tc.tile_pool(name="mpool", bufs=2) as mpool,
            tc.tile_pool(name="gps", bufs=8, space="PSUM") as gps,
        ):
            ones = consts.tile([1, 128], mybir.dt.float16)
            nc.vector.memset(ones, 1.0)
            msel = consts.tile([128, NBLK * 8], mybir.dt.float32)
            # all 6 samples' bias rows in one partition-0 tile, one DMA
            sqn_all = consts.tile([1, UPC * C], mybir.dt.float16)
            nc.sync.dma_start(
                out=sqn_all, in_=sqn_d.ap().rearrange("s c -> (s c)").unsqueeze(0)
            )

            def pipeline_body(_iv=None):
                xparts_cached = None
                for s in range(UPC):
                    if "dma" in ablate and xparts_cached is not None:
                        xparts = xparts_cached
                    else:
                        xparts = []
                        for d in range(DMA_SPLIT):
                            xp = xpool.tile(
                                [128, kper, C], mybir.dt.float16, tag="xts"
                            )
                            nc.sync.dma_start(out=xp, in_=xt_view[s, :, d])
                            xparts.append(xp)
                        xparts_cached = xparts

                    sqn = sqn_all[:, s * C : (s + 1) * C]

                    for I in range(RBLK):
                        blk = s * RBLK + I
                        g_ps = gps.tile([128, C], mybir.dt.float32, tag="g")
                        # K=1 bias row: m += ones^T @ sqn (broadcast along rows)
                        nc.tensor.matmul(
                            out=g_ps, lhsT=ones, rhs=sqn, start=True, stop=False
                        )
                        nkc = 1 if "mm" in ablate else KCHUNKS
                        for k in range(nkc):
                            xp = xparts[k // kper]
                            kk = k % kper
                            nc.tensor.matmul(
                                out=g_ps,
                                lhsT=xp[:, kk, 128 * I : 128 * (I + 1)],
                                rhs=xp[:, kk, :],
                                start=False,
                                stop=(k == nkc - 1),
                            )
                        m = mpool.tile([128, C], mybir.dt.float32, tag="m")
                        nc.scalar.copy(out=m, in_=g_ps)
                        nrounds = 1 if "sel" in ablate else ROUNDS
                        for r in range(nrounds):
                            if r == nrounds - 1:
                                nc.vector.max(
                                    out=msel[:, blk * 8 : blk * 8 + 8], in_=m
                                )
                            else:
                                mx = mpool.tile([128, 8], mybir.dt.float32, tag="mx")
                                nc.vector.max(out=mx, in_=m)
                                nc.vector.match_replace(
                                    out=m, in_to_replace=mx, in_values=m,
                                    imm_value=-1e30,
                                )

            if loop_n is not None:
                with tc.For_i(0, loop_n, 1) as _iv:
                    pipeline_body(_iv)
            else:
                for _rep in range(repeat):
                    pipeline_body()

            nc.sync.dma_start(out=msel_d.ap(), in_=msel)

    nc.compile()
    return nc


_PROGRAM = None


def kernel(feat0, feat1, feat2):
    global _PROGRAM
    feats = np.stack(
        [np.asarray(f).reshape(B, C, D) for f in (feat0, feat1, feat2)]
    ).reshape(UNITS, C, D)

    # sq in fp64 (host); device accumulates fp16(2048 - sq/2) via a K=1
    # matmul row so PSUM holds m = G - sq_j/2 + 2048 directly
    sq64 = np.einsum(
        "ucd,ucd->uc", feats, feats, dtype=np.float64, casting="safe"
    )
    sqn16 = (2048.0 - sq64 / 2.0).astype(np.float16)

    # X^T in [128, 32, 512] chunk layout, fp16
    # xt[u, p, k, c] = X[c, 128k + p]
    xt = np.ascontiguousarray(
        feats.astype(np.float16)
        .transpose(0, 2, 1)  # [U, D, C]
        .reshape(UNITS, KCHUNKS, 128, C)
        .transpose(0, 2, 1, 3)  # [U, 128, K, C]
        .reshape(UNITS, 128, KCHUNKS * C)
    )

    if _PROGRAM is None:
        _PROGRAM = _build_program()
    nc = _PROGRAM
    in_maps = [
        {
            "xt": xt[c * UPC : (c + 1) * UPC],
            "sqn": sqn16[c * UPC : (c + 1) * UPC],
        }
        for c in range(N_CORES)
    ]
    out = run_bass_kernel_spmd(
        nc, in_maps, core_ids=list(range(N_CORES)), trace=TRACE
    )
    _LAST.clear()
    _LAST["results"] = out

    # msel[p, (s*4+I)*8 + j] = (j+1)-th largest m of row (I*128+p) of unit s
    m52 = np.empty((UNITS, C), dtype=np.float64)
    for c in range(N_CORES):
        sel = out.results[c]["msel"].reshape(128, UPC, RBLK, 8)[:, :, :, SEL_COL]
        m52[c * UPC : (c + 1) * UPC] = sel.transpose(1, 2, 0).reshape(UPC, C)

    # device m = G - sq_j/2 + 2048 (with sqn's fp16 rounding folded into
    # both ranking and value, consistently)
    d2 = sq64 + 4096.0 - 2.0 * m52
    r = np.sqrt(np.clip(d2, 0.0, None))  # [UNITS, C]
    _LAST["r"] = r
    sums = r.reshape(N_TENSORS, B * C).sum(axis=1)
    e = np.log(sums + 1.0)
    deltas = np.array([e[1] - e[0], e[2] - e[1]])
    var = deltas.var(ddof=1)
    return np.asarray(var, dtype=np.float32)
